# revision 38
# baseline (speedup 1.0000x reference)
"""Trainium2 Bass kernel for nn_ContactPredictionHead.

Reference computation (B=2, L=2048, D=1536, T=2):
    Wp, Wd = W[:, :D], W[:, D:]
    prod[b,i,j,t] = sum_d h[b,i,d] * Wp[t,d] * h[b,j,d]
    diff[b,i,j,t] = (h@Wd.T)[b,i,t] - (h@Wd.T)[b,j,t]
    out = symmetrize(prod + diff + bias)

Key identity: prod is symmetric in (i,j) and diff is antisymmetric, so the
symmetrization leaves   out[b,i,j,t] = prod[b,i,j,t] + bias[t]   exactly —
a weighted Gram matrix.  Only the upper triangle is computed on device; the
host mirrors the strict lower triangle.

Sharding: 4 cores per batch item.  The 16 row-blocks (128 rows each) of a
batch's L x L Gram matrix are dealt by a Latin square: core cc's stationary
slot s holds row-block I = 4s + ((s+cc)%4).  Slot s covers its arc
[128I, 2048) as one cc-dependent "partial" group [128I, 512(s+1)) plus
(3-s) full 512-col "static" groups — so every core computes exactly
4352 moving columns per t (the balanced ideal; the old aligned scheme did
5120).  Static groups are identical on all cores; the partial groups'
offsets/sizes live in a 4-way partition-id branch on the Tensor engine
only (all other engines run straight-line code: PSUM accs are padded to
512 and the host slices each group's valid columns).

Stationary operands are built on device from htall (each slot's 128 rows
are a j-window of the streamed chunks, so no extra HBM traffic): a small
4-variant branch on the Vector engine multiplies the slot's window by Wp.
All tensors stream as bfloat16 (PSUM accumulates fp32), halving DMA
volume vs fp32 at the same PE rate.
"""
import sys

sys.path.insert(0, "/opt/trn_rl_repo")

import numpy as np
import ml_dtypes

BF16 = ml_dtypes.bfloat16

B, L, D, T = 2, 2048, 1536, 2
NCORES = 8
CPB = NCORES // B     # cores per batch item = 4
NK = D // 128         # contraction k-tiles = 12
NJ = 512              # j columns per full matmul (one PSUM bank of fp32)
NNB = L // NJ         # j chunks = 4
NS = 4                # stationary row slots per core (128 rows each)


def row_of(s, cc):
    """Global 128-row block held by slot s on a core with variant cc."""
    return 4 * s + (s + cc) % 4


def groups_of(cc):
    """Schedule (shared shape, variant-dependent geometry): list of
    (I, colstart, F) in emission order; 20 groups = [partial t0, t1,
    statics (s<v) t0, t1] per chunk phase v."""
    gs = []
    for v in range(NNB):
        q = (v + cc) % 4
        for _t in range(T):
            gs.append((row_of(v, cc), NJ * v + 128 * q, NJ - 128 * q))
        for s in range(v):
            for _t in range(T):
                gs.append((row_of(s, cc), NJ * v, NJ))
    return gs


NG = len(groups_of(0))    # 20
# DMA k-parts per chunk: chunks 0/1 are quartered so the first matmul and
# phase 1 start on a quarter-chunk; still-finer splits lose more to the
# ~0.7us fixed issue cost per DMA than the earlier starts gain.
PARTS = [4, 4, 2, 2]

_CACHE = {}


def _get_nc():
    if "nc" in _CACHE:
        return _CACHE["nc"]
    import concourse.tile as tile
    from concourse.tile_rust import add_dep_helper
    from concourse import bacc, mybir

    f32, bf16 = mybir.dt.float32, mybir.dt.bfloat16
    nc = bacc.Bacc("TRN2", target_bir_lowering=False, debug=False,
                   num_devices=NCORES, enable_partition_id=True,
                   enable_asserts=False)
    ht_d = nc.dram_tensor("ht", [D, L], bf16, kind="ExternalInput")
    wp_d = nc.dram_tensor("wp", [128, T * NK], bf16, kind="ExternalInput")
    out_d = nc.dram_tensor("out", [NG, 128, NJ], bf16, kind="ExternalOutput")

    with tile.TileContext(nc) as tc:
        with tc.tile_pool(name="big", bufs=1) as big, \
             tc.tile_pool(name="st", bufs=4) as stp, \
             tc.tile_pool(name="ps", bufs=4, space="PSUM") as psp, \
             tc.tile_pool(name="psw", bufs=1, space="PSUM") as psw, \
             tc.tile_pool(name="psh", bufs=2, space="PSUM") as psh:
            # wt[p, t*NK+k] = Wp[t, 128k+p] (pre-gathered on the host)
            wt = big.tile([128, T * NK], bf16, name="wt")
            # hst[p, s, t, k, r] = ht[128k+p, 128*row_of(s,cc)+r] * Wp[t,128k+p]
            hst = big.tile([128, NS, T, NK, 128], bf16, name="hst")
            # htall[p, k, j] = ht[128k+p, j]  (canonical, un-rolled)
            htall = big.tile([128, NK, L], bf16, name="htall")

            # Partition-id register loads cost ~1.4us of queue time each —
            # issue them first so they overlap the framework preamble and
            # DMA issues instead of delaying the branch evaluations later.
            pid = nc.tensor.partition_id()
            cc = pid % 4
            vcc = nc.vector.partition_id() % 4

            nc.scalar.dma_start(wt[:], wp_d.ap())
            # Warm the PE clock (HAM un-throttles after ~3.4 us of activity)
            # with throwaway matmuls on a locally-initialized scratch tile —
            # no DMA dependency, so warmup starts during the preamble.  The
            # memset rides gpsimd so the vector queue stays clear for the
            # pid load + stationary preps.
            wdum = big.tile([128, NJ], bf16, name="wdum")
            nc.gpsimd.memset(wdum[:], 0.0)
            wacc = psw.tile([128, NJ], f32, name="wacc")
            for _ in range(16):
                nc.tensor.matmul(wacc[:, 0:128], wdum[:, 0:128],
                                 wdum[:, 0:128], start=True, stop=True)

            # ht chunks land in fixed order so the PE can start on chunk 0
            # while the rest stream in.  Parts alternate between the two
            # HWDGE rings (sync / scalar) to raise aggregate DMA bandwidth;
            # chunk 0 is quartered for an earlier first matmul.
            prev = None
            for v in range(NNB):
                nparts = PARTS[v]
                kq = NK // nparts
                for h in range(nparts):
                    dma = nc.sync.dma_start(
                        htall[:, h * kq:(h + 1) * kq, v * NJ:(v + 1) * NJ],
                        ht_d.ap()[h * kq * 128:(h + 1) * kq * 128,
                                  v * NJ:(v + 1) * NJ]
                        .rearrange("(k p) j -> p k j", p=128))
                    if prev is not None:
                        add_dep_helper(dma.ins, prev.ins, sync=False,
                                       reason="ht chunks stream in j order")
                    prev = dma

            def fourway(ccreg, fn):
                with tc.If(ccreg <= 1) as c1:
                    with tc.If(ccreg == 0) as c2:
                        fn(0)
                    with c2.Else():
                        fn(1)
                with c1.Else():
                    with tc.If(ccreg == 2) as c3:
                        fn(2)
                    with c3.Else():
                        fn(3)

            # Stationary prep: slot s's rows are htall cols
            # [128*row_of(s,cc), +128) — already streamed with chunk s.
            # Muls are split to match the chunk DMA k-parts.  All preps are
            # emitted up front (the vector engine does nothing else), so
            # prep(s) runs as soon as chunk s lands, overlapped with the
            # phase s-1 matmuls.
            def emit_preps(ccval):
                for s in range(NS):
                    base = 128 * row_of(s, ccval)
                    nparts = PARTS[s]
                    kq = NK // nparts
                    for h in range(nparts):
                        ks = slice(h * kq, (h + 1) * kq)
                        for t in range(T):
                            scale = (wt[:, t * NK + h * kq:
                                        t * NK + (h + 1) * kq]
                                     .unsqueeze(2)
                                     .broadcast_to([128, kq, 128]))
                            nc.vector.tensor_mul(
                                hst[:, s, t, ks],
                                htall[:, ks, base:base + 128], scale)

            fourway(vcc, emit_preps)

            def emit_partial(v, accs, ccv):
                q = (v + ccv) % 4
                off, fw = NJ * v + 128 * q, NJ - 128 * q
                for t in range(T):
                    for k in range(NK):
                        nc.tensor.matmul(
                            accs[t][:, 0:fw], hst[:, v, t, k],
                            htall[:, k, off:off + fw],
                            start=(k == 0), stop=(k == NK - 1))

            def emit_static(s, v, acc, t):
                for k in range(NK):
                    nc.tensor.matmul(
                        acc[:], hst[:, s, t, k],
                        htall[:, k, v * NJ:(v + 1) * NJ],
                        start=(k == 0), stop=(k == NK - 1))

            def store(gi, acc, t):
                # Copies on scalar: the vector queue must stay free for the
                # stationary preps (a copy would trap prep(s+1) behind the
                # phase-s matmul completion).  The last two stores split
                # across vector+gpsimd / scalar+scalar so the tail chains
                # run in parallel (vector is idle by then).
                st = stp.tile([128, NJ], bf16, name="st", tag="st")
                if gi == NG - 2:
                    nc.vector.tensor_copy(st[:], acc[:])
                    nc.gpsimd.dma_start(out_d.ap()[gi], st[:])
                else:
                    nc.scalar.copy(st[:], acc[:])
                    out_eng = nc.scalar if gi == NG - 1 else nc.gpsimd
                    out_eng.dma_start(out_d.ap()[gi], st[:])

            gi = 0
            for v in range(NNB):
                accs = [psp.tile([128, NJ], f32, name="acc", tag="acc")
                        for _t in range(T)]
                with tc.If(cc <= 1) as c1:
                    with tc.If(cc == 0) as c2:
                        emit_partial(v, accs, 0)
                    with c2.Else():
                        emit_partial(v, accs, 1)
                with c1.Else():
                    with tc.If(cc == 2) as c3:
                        emit_partial(v, accs, 2)
                    with c3.Else():
                        emit_partial(v, accs, 3)
                for t in range(T):
                    store(gi, accs[t], t)
                    gi += 1
                for s in range(v):
                    for t in range(T):
                        if gi == NG - 1:
                            # Final group split into two 256-col halves: the
                            # first half's store chain hides inside the
                            # second half's matmuls, and the last chain
                            # (vector copy + gpsimd DMA) runs in parallel
                            # with nothing behind it.
                            for hh in range(2):
                                acch = psh.tile([128, NJ // 2], f32,
                                                name="acch", tag="acch")
                                off = v * NJ + hh * (NJ // 2)
                                for k in range(NK):
                                    nc.tensor.matmul(
                                        acch[:], hst[:, s, t, k],
                                        htall[:, k, off:off + NJ // 2],
                                        start=(k == 0), stop=(k == NK - 1))
                                sth = stp.tile([128, NJ // 2], bf16,
                                               name="sth", tag="sth")
                                dst = out_d.ap()[gi][:, hh * (NJ // 2):
                                                     (hh + 1) * (NJ // 2)]
                                if hh == 0:
                                    nc.scalar.copy(sth[:], acch[:])
                                    nc.scalar.dma_start(dst, sth[:])
                                else:
                                    # Final chain rides vector + sync: both
                                    # queues are idle by now and sync's
                                    # teardown drain is ~8ns (gpsimd's is
                                    # ~2.7us and would serialize at the end).
                                    nc.vector.tensor_copy(sth[:], acch[:])
                                    nc.sync.dma_start(dst, sth[:])
                        else:
                            acc = psp.tile([128, NJ], f32, name="acc",
                                           tag="acc")
                            emit_static(s, v, acc, t)
                            store(gi, acc, t)
                        gi += 1
                if v == 0:
                    # Wide keep-warm matmuls bridge the chunk-1 wait with
                    # continuous PE activity so the HAM monitor never
                    # re-throttles the clock.
                    for _ in range(8):
                        nc.tensor.matmul(wacc[0:64, :], wdum[:, 0:64],
                                         wdum[:], start=True, stop=True)
    nc.compile()
    _CACHE["nc"] = nc
    return nc


def make_in_maps(h, W):
    # wp[p, t*NK+k] = Wp[t, 128k+p]
    wp = np.ascontiguousarray(
        W[:, :D].reshape(T, NK, 128).transpose(2, 0, 1)
        .reshape(128, T * NK)).astype(BF16)
    hts = [np.ascontiguousarray(h[bi].T).astype(BF16) for bi in range(B)]
    in_maps = [{"ht": hts[c // CPB], "wp": wp} for c in range(NCORES)]
    return in_maps


def kernel(hidden_states, W, b):
    from concourse.bass_utils import run_bass_kernel_spmd

    h = np.ascontiguousarray(hidden_states, dtype=np.float32)
    W = np.asarray(W, dtype=np.float32)
    bias = np.asarray(b, dtype=np.float32)
    nc = _get_nc()

    res = run_bass_kernel_spmd(nc, make_in_maps(h, W),
                               core_ids=list(range(NCORES)))
    full = np.empty((B, L, L, T), np.float32)
    for c in range(NCORES):
        bi, cc = c // CPB, c % CPB
        blocks = np.asarray(res.results[c]["out"]).astype(np.float32)
        for gi, (I, colstart, fw) in enumerate(groups_of(cc)):
            t = gi % T
            rows = slice(128 * I, 128 * I + 128)
            full[bi, rows, colstart:colstart + fw, t] = blocks[gi, :, 0:fw]
    # Mirror: keep computed j >= i, take j < i from the transpose.
    idx = np.arange(L)
    mask = (idx[None, :] >= idx[:, None])[None, :, :, None]
    out = np.where(mask, full, full.transpose(0, 2, 1, 3))
    if np.any(bias != 0):
        out += bias
    return out


# revision 39
# speedup vs baseline: 1.0222x; 1.0222x over previous
"""Trainium2 Bass kernel for nn_ContactPredictionHead.

Reference computation (B=2, L=2048, D=1536, T=2):
    Wp, Wd = W[:, :D], W[:, D:]
    prod[b,i,j,t] = sum_d h[b,i,d] * Wp[t,d] * h[b,j,d]
    diff[b,i,j,t] = (h@Wd.T)[b,i,t] - (h@Wd.T)[b,j,t]
    out = symmetrize(prod + diff + bias)

Key identity: prod is symmetric in (i,j) and diff is antisymmetric, so the
symmetrization leaves   out[b,i,j,t] = prod[b,i,j,t] + bias[t]   exactly —
a weighted Gram matrix.  Only the upper triangle is computed on device; the
host mirrors the strict lower triangle.

Sharding: 4 cores per batch item.  The 16 row-blocks (128 rows each) of a
batch's L x L Gram matrix are dealt by a Latin square: core cc's stationary
slot s holds row-block I = 4s + ((s+cc)%4).  Slot s covers its arc
[128I, 2048) as one cc-dependent "partial" group [128I, 512(s+1)) plus
(3-s) full 512-col "static" groups — so every core computes exactly
4352 moving columns per t (the balanced ideal; the aligned scheme needs
5120).  Static groups are identical on all cores; the partial groups'
offsets/sizes live in a 4-way partition-id branch on the Tensor engine
only (all other engines run straight-line code: PSUM accs are padded to
512 and the host slices each group's valid columns).

Phases run in REVERSE chunk order (3,2,1,0): per-phase matmul work
(18.6/13.4/8.3/3.2 us) then always exceeds the per-chunk stream time, so
the PE can never starve on the input stream — no mid-run clock throttle,
robust to HBM contention.  This requires the stationary windows to arrive
independently of the chunks: the host gathers each core's four 128-row
slot windows into a separate small input ("hw", 0.4MB/slot), which also
makes the weight-multiply preps straight-line (per-core-ness lives in the
data, not the addressing).

All tensors stream as bfloat16 (PSUM accumulates fp32), halving DMA
volume vs fp32 at the same PE rate.
"""
import sys

sys.path.insert(0, "/opt/trn_rl_repo")

import numpy as np
import ml_dtypes

BF16 = ml_dtypes.bfloat16

B, L, D, T = 2, 2048, 1536, 2
NCORES = 8
CPB = NCORES // B     # cores per batch item = 4
NK = D // 128         # contraction k-tiles = 12
NJ = 512              # j columns per full matmul (one PSUM bank of fp32)
NNB = L // NJ         # j chunks = 4
NS = 4                # stationary row slots per core (128 rows each)

PHASES = [3, 2, 1, 0]           # chunk/phase order (heaviest work first)
PREPS = [3, 0, 1, 2]            # slot-window arrival + prep order
# DMA k-parts per chunk: chunk 3 (consumed first) is quartered so the
# first matmuls start on a quarter-chunk; finer splits lose more to the
# ~0.7us fixed issue cost per DMA than the earlier starts gain.
PARTS = {3: 4, 2: 2, 1: 2, 0: 2}


def row_of(s, cc):
    """Global 128-row block held by slot s on a core with variant cc."""
    return 4 * s + (s + cc) % 4


def groups_of(cc):
    """Schedule (shared shape, variant-dependent geometry): list of
    (I, colstart, F) in emission order; 20 groups = [partial t0, t1,
    statics (s<v) t0, t1] per chunk phase v, phases in PHASES order."""
    gs = []
    for v in PHASES:
        q = (v + cc) % 4
        for _t in range(T):
            gs.append((row_of(v, cc), NJ * v + 128 * q, NJ - 128 * q))
        for s in range(v):
            for _t in range(T):
                gs.append((row_of(s, cc), NJ * v, NJ))
    return gs


NG = len(groups_of(0))    # 20

_CACHE = {}


def _get_nc():
    if "nc" in _CACHE:
        return _CACHE["nc"]
    import concourse.tile as tile
    from concourse.tile_rust import add_dep_helper
    from concourse import bacc, mybir

    f32, bf16 = mybir.dt.float32, mybir.dt.bfloat16
    nc = bacc.Bacc("TRN2", target_bir_lowering=False, debug=False,
                   num_devices=NCORES, enable_partition_id=True,
                   enable_asserts=False)
    ht_d = nc.dram_tensor("ht", [D, L], bf16, kind="ExternalInput")
    hw_d = nc.dram_tensor("hw", [D, NS * 128], bf16, kind="ExternalInput")
    wp_d = nc.dram_tensor("wp", [128, T * NK], bf16, kind="ExternalInput")
    out_d = nc.dram_tensor("out", [NG, 128, NJ], bf16, kind="ExternalOutput")

    with tile.TileContext(nc) as tc:
        with tc.tile_pool(name="big", bufs=1) as big, \
             tc.tile_pool(name="st", bufs=4) as stp, \
             tc.tile_pool(name="ps", bufs=4, space="PSUM") as psp, \
             tc.tile_pool(name="psw", bufs=1, space="PSUM") as psw:
            # wt[p, t*NK+k] = Wp[t, 128k+p] (pre-gathered on the host)
            wt = big.tile([128, T * NK], bf16, name="wt")
            # hst[p, s, t, k, r] = hw[128k+p, 128s+r] * Wp[t, 128k+p]
            hst = big.tile([128, NS, T, NK, 128], bf16, name="hst")
            # htw[p, k, 128s+r] = hw[128k+p, 128s+r]  (slot windows)
            htw = big.tile([128, NK, NS * 128], bf16, name="htw")
            # htall[p, k, j] = ht[128k+p, j]  (canonical, un-rolled)
            htall = big.tile([128, NK, L], bf16, name="htall")

            # Partition-id register load costs ~1.4us of queue time — issue
            # it first so it overlaps the framework preamble and DMA issues
            # instead of delaying the branch evaluations later.
            pid = nc.tensor.partition_id()
            cc = pid % 4

            nc.scalar.dma_start(wt[:], wp_d.ap())
            # Warm the PE clock (HAM un-throttles after ~3.4 us of activity)
            # with throwaway matmuls on a locally-initialized scratch tile —
            # no DMA dependency, so warmup starts during the preamble.  The
            # memset rides gpsimd so the vector queue stays clear for the
            # stationary preps.
            wdum = big.tile([128, NJ], bf16, name="wdum")
            nc.gpsimd.memset(wdum[:], 0.0)
            wacc = psw.tile([128, NJ], f32, name="wacc")
            for _ in range(16):
                nc.tensor.matmul(wacc[:, 0:128], wdum[:, 0:128],
                                 wdum[:, 0:128], start=True, stop=True)

            # Input stream, strictly ordered on the sync ring in consumption
            # order: slot-3 window, chunk 3, remaining slot windows, then
            # chunks 2, 1, 0.
            prev = None

            def chain(dma):
                nonlocal prev
                if prev is not None:
                    add_dep_helper(dma.ins, prev.ins, sync=False,
                                   reason="input stream in consumption order")
                prev = dma

            def win_dma(s):
                chain(nc.sync.dma_start(
                    htw[:, :, 128 * s:128 * (s + 1)],
                    hw_d.ap()[:, 128 * s:128 * (s + 1)]
                    .rearrange("(k p) r -> p k r", p=128)))

            def chunk_dma(v):
                nparts = PARTS[v]
                kq = NK // nparts
                for h in range(nparts):
                    chain(nc.sync.dma_start(
                        htall[:, h * kq:(h + 1) * kq, v * NJ:(v + 1) * NJ],
                        ht_d.ap()[h * kq * 128:(h + 1) * kq * 128,
                                  v * NJ:(v + 1) * NJ]
                        .rearrange("(k p) j -> p k j", p=128)))

            win_dma(PREPS[0])
            chunk_dma(PHASES[0])
            for s in PREPS[1:]:
                win_dma(s)
            for v in PHASES[1:]:
                chunk_dma(v)

            # Stationary prep (straight-line; per-core rows arrive via hw):
            # hst[:, s, t] = htw window s  *  Wp[t] broadcast along rows.
            for s in PREPS:
                for t in range(T):
                    scale = (wt[:, t * NK:(t + 1) * NK].unsqueeze(2)
                             .broadcast_to([128, NK, 128]))
                    nc.vector.tensor_mul(
                        hst[:, s, t], htw[:, :, 128 * s:128 * (s + 1)],
                        scale)

            def emit_partial(v, accs, ccv):
                q = (v + ccv) % 4
                off, fw = NJ * v + 128 * q, NJ - 128 * q
                for t in range(T):
                    for k in range(NK):
                        nc.tensor.matmul(
                            accs[t][:, 0:fw], hst[:, v, t, k],
                            htall[:, k, off:off + fw],
                            start=(k == 0), stop=(k == NK - 1))

            def emit_static(s, v, acc, t):
                for k in range(NK):
                    nc.tensor.matmul(
                        acc[:], hst[:, s, t, k],
                        htall[:, k, v * NJ:(v + 1) * NJ],
                        start=(k == 0), stop=(k == NK - 1))

            def store(gi, acc):
                # Copies on scalar: the vector queue must stay free for the
                # stationary preps (a copy would trap a later prep behind a
                # matmul completion).  The last two groups (the v=0
                # partials) route around gpsimd, whose teardown drain is
                # ~2.7us: t0's chain hides inside t1's matmuls; t1's store
                # splits into two halves on parallel vector+sync /
                # scalar+scalar chains (sync's drain is ~8ns).
                if gi == NG - 2:
                    st = stp.tile([128, NJ], bf16, name="st", tag="st")
                    nc.vector.tensor_copy(st[:], acc[:])
                    nc.sync.dma_start(out_d.ap()[gi], st[:])
                elif gi == NG - 1:
                    for hh in range(2):
                        sth = stp.tile([128, NJ // 2], bf16, name="sth",
                                       tag="sth")
                        half = slice(hh * (NJ // 2), (hh + 1) * (NJ // 2))
                        if hh == 0:
                            nc.scalar.copy(sth[:], acc[:, half])
                            nc.scalar.dma_start(out_d.ap()[gi][:, half],
                                                sth[:])
                        else:
                            nc.vector.tensor_copy(sth[:], acc[:, half])
                            nc.sync.dma_start(out_d.ap()[gi][:, half],
                                              sth[:])
                else:
                    st = stp.tile([128, NJ], bf16, name="st", tag="st")
                    nc.scalar.copy(st[:], acc[:])
                    nc.gpsimd.dma_start(out_d.ap()[gi], st[:])

            gi = 0
            for v in PHASES:
                accs = [psp.tile([128, NJ], f32, name="acc", tag="acc")
                        for _t in range(T)]
                with tc.If(cc <= 1) as c1:
                    with tc.If(cc == 0) as c2:
                        emit_partial(v, accs, 0)
                    with c2.Else():
                        emit_partial(v, accs, 1)
                with c1.Else():
                    with tc.If(cc == 2) as c3:
                        emit_partial(v, accs, 2)
                    with c3.Else():
                        emit_partial(v, accs, 3)
                for t in range(T):
                    store(gi, accs[t])
                    gi += 1
                for s in range(v):
                    for t in range(T):
                        acc = psp.tile([128, NJ], f32, name="acc", tag="acc")
                        emit_static(s, v, acc, t)
                        store(gi, acc)
                        gi += 1
    nc.compile()
    _CACHE["nc"] = nc
    return nc


def make_in_maps(h, W):
    # wp[p, t*NK+k] = Wp[t, 128k+p]
    wp = np.ascontiguousarray(
        W[:, :D].reshape(T, NK, 128).transpose(2, 0, 1)
        .reshape(128, T * NK)).astype(BF16)
    hts = [np.ascontiguousarray(h[bi].T).astype(BF16) for bi in range(B)]
    in_maps = []
    for c in range(NCORES):
        bi, cc = c // CPB, c % CPB
        hw = np.concatenate(
            [hts[bi][:, 128 * row_of(s, cc):128 * row_of(s, cc) + 128]
             for s in range(NS)], axis=1)
        in_maps.append({"ht": hts[bi], "hw": np.ascontiguousarray(hw),
                        "wp": wp})
    return in_maps


def kernel(hidden_states, W, b):
    from concourse.bass_utils import run_bass_kernel_spmd

    h = np.ascontiguousarray(hidden_states, dtype=np.float32)
    W = np.asarray(W, dtype=np.float32)
    bias = np.asarray(b, dtype=np.float32)
    nc = _get_nc()

    res = run_bass_kernel_spmd(nc, make_in_maps(h, W),
                               core_ids=list(range(NCORES)))
    full = np.empty((B, L, L, T), np.float32)
    for c in range(NCORES):
        bi, cc = c // CPB, c % CPB
        blocks = np.asarray(res.results[c]["out"]).astype(np.float32)
        for gi, (I, colstart, fw) in enumerate(groups_of(cc)):
            t = gi % T
            rows = slice(128 * I, 128 * I + 128)
            full[bi, rows, colstart:colstart + fw, t] = blocks[gi, :, 0:fw]
    # Mirror: keep computed j >= i, take j < i from the transpose.
    idx = np.arange(L)
    mask = (idx[None, :] >= idx[:, None])[None, :, :, None]
    out = np.where(mask, full, full.transpose(0, 2, 1, 3))
    if np.any(bias != 0):
        out += bias
    return out
